# revision 9
# baseline (speedup 1.0000x reference)
"""Detail-loss kernel for TRN2 (8 NeuronCores), v3.

Reference computation (algebraically reduced):
  views = reshape(inputs, (98, 3, 256, 256)); d = infer - ref
  S[n] = sum_c d[n, c]                       (per-view 256x256 plane)
  loss = ( sum |S[n,h,w+1] - S[n,h,w-1]|     (zero-padded outside)
         + sum |S[n,h+1,w] - S[n,h-1,w]| ) / (4 * 98 * 258 * 256)

Sharding: 98 views padded to 104, 13 views per core (zero views add 0).

v3 changes vs v2 (41.8us):
  * DMA rebalanced: views 0-4 on gpsimd(SWDGE), 5-8 on sync, 9-12 on
    scalar queues as 2-view-group transfers (128 x 6KB descriptors).
    v2 put 9/13 views on gpsimd -> 7us single-queue DMA tail.
  * PE warmup: dummy matmuls on junk data at kernel start lift the
    PE_HAM clock gate (1.2 -> 2.4 GHz) before real data arrives, so
    real matmuls run at the 216ns warm cadence instead of ~430ns.
  * gw computed directly from PSUM S on DVE (interior diff + edge-col
    copy), removing the S-copy -> gw chain; S-copy (needed only as the
    gh matmul moving operand) runs in parallel on ACT.
  * per-pair abs-accumulate ops alternate DVE/ACT; some mid-kernel gw
    subtracts route via SBUF on gpsimd to relieve DVE.
Host: sum partials in float64, scale.
"""
import numpy as np
import ml_dtypes
import concourse.bass as bass
import concourse.mybir as mybir
from concourse import bacc
from concourse.tile import TileContext
from concourse.bass_utils import run_bass_kernel_spmd

N_CORES = 8
V = 13                       # views per core (98 -> 104 padded)
C, H, W = 3, 256, 256
SCALE = 1.0 / (4.0 * 98.0 * 258.0 * 256.0)
NPAIR = 7
NCOL = 2 * NPAIR             # 7 gw cols + 7 gh cols

# DMA groups: (queue, view_lo, n_views). Issue order per queue is list
# order; process order (below) interleaves queues by expected arrival.
DMA_GROUPS = [
    ("g", 0, 2), ("g", 2, 2), ("g", 4, 1),   # gpsimd / SWDGE
    ("s", 5, 2), ("s", 7, 2),                # sync   / HWDGE ring 0
    ("c", 9, 2), ("c", 11, 2),               # scalar / HWDGE ring 1
]
# pair index -> DMA_GROUPS index, in processing order (tuned to arrival)
PROC = [0, 5, 3, 1, 6, 4, 2]

N_WARM = 10                  # PE warmup matmuls (512 cols each)

# engine assignment per pair: S-copy engine, gw-abs, gh-abs ('a'=ACT,
# 'v'=DVE); gw-sub source ('p'=PSUM on DVE, 's'=SBUF stp on gpsimd)
COPY_ENG = "aavaava"
GWABS_ENG = "avavava"
GHABS_ENG = "vavavav"
GWSUB_SRC = "ppsspppp"[:NPAIR]

_cache = {}


def _weights():
    I = np.eye(128, dtype=np.float32)
    E = (np.eye(128) - np.eye(128, k=1)).astype(np.float32)   # out[o]=in[o]-in[o-1]
    O = (np.eye(128, k=-1) - np.eye(128)).astype(np.float32)  # out[o]=in[o+1]-in[o]
    wpair = np.stack([I, -I], axis=1)  # [128, 2, 128] DoubleRow stationary
    weo = np.stack([E, O], axis=1)     # [128, 2, 128]
    return wpair, weo


def _build():
    if "nc" in _cache:
        return _cache["nc"]
    f32 = mybir.dt.float32
    bf16 = mybir.dt.bfloat16
    f8 = mybir.dt.float8e4
    AluOp = mybir.AluOpType
    Act = mybir.ActivationFunctionType
    DR = mybir.MatmulPerfMode.DoubleRow

    nc = bacc.Bacc(None, target_bir_lowering=False)
    x = nc.declare_dram_parameter("x", [128, V, C, 2, 512], f8, isOutput=False)
    wp = nc.declare_dram_parameter("wp", [128, 2, 128], f8, isOutput=False)
    we = nc.declare_dram_parameter("we", [128, 2, 128], bf16, isOutput=False)
    y = nc.declare_dram_parameter("y", [128, NCOL], f32, isOutput=True)

    with TileContext(nc) as tc:
        with (
            tc.tile_pool(name="wpool", bufs=1) as wpool,
            tc.tile_pool(name="jp", bufs=1) as jpool,
            tc.tile_pool(name="xp", bufs=1) as xpool,
            tc.tile_pool(name="sp", bufs=3) as spool,
            tc.tile_pool(name="gp", bufs=2) as gpool,
            tc.tile_pool(name="zp", bufs=2) as zpool,
            tc.tile_pool(name="cp", bufs=2) as cpool,
            tc.tile_pool(name="ap", bufs=1) as apool,
            tc.tile_pool(name="psS", bufs=2, space="PSUM") as psSp,
            tc.tile_pool(name="psG", bufs=2, space="PSUM") as psGp,
        ):
            # ---- weight DMAs first on sync (HWDGE, fast first-byte)
            wpt = wpool.tile([128, 2, 128], f8)
            wet = wpool.tile([128, 2, 128], bf16)
            nc.sync.dma_start(out=wpt[:], in_=wp[:])
            nc.sync.dma_start(out=wet[:], in_=we[:])

            # ---- per-group input DMAs
            qmap = {"g": nc.gpsimd, "s": nc.sync, "c": nc.scalar}
            xts = []
            for gi, (q, lo, n) in enumerate(DMA_GROUPS):
                xt = xpool.tile([128, n, C, 2, 512], f8, name=f"xg{gi}")
                qmap[q].dma_start(out=xt[:], in_=x[:, lo : lo + n])
                xts.append(xt)

            # ---- PE warmup: junk matmuls lift the HAM clock gate while
            # DMA streams in. Junk weights/moving data from a memset tile;
            # output overwrites a scratch PSUM tile nobody reads.
            junk = jpool.tile([128, 2, 640], f8)
            nc.vector.memset(junk[:], 0.0)
            warm = psGp.tile([128, 4, 256], f32, name="warm", tag="psg")
            wview = warm[:].rearrange("p a b -> p (a b)")
            for _ in range(N_WARM):
                nc.tensor.matmul(
                    wview[:, 0:512], junk[:, :, 0:128], junk[:, :, 128:640],
                    start=True, stop=True, perf_mode=DR,
                )

            acc = apool.tile([128, NCOL], f32)

            sts = {}   # pair k -> stp tile (SBUF S, gh moving operand)
            pss_of = {}

            def emit_gh(k, n):
                stp = sts[k]
                psg = psGp.tile([128, 4, 256], f32, name="psg", tag="psg")
                nc.tensor.matmul(
                    psg[:, 0:n, :], wet[:, 0, :], stp[:, 0:n, 1, :],
                    start=True, stop=True,
                )
                nc.tensor.matmul(
                    psg[:, n : 2 * n, :], wet[:, 1, :], stp[:, 0:n, 0, :],
                    start=True, stop=True,
                )
                hcol = acc[:, NPAIR + k : NPAIR + k + 1]
                if GHABS_ENG[k] == "a":
                    scg = cpool.tile([128, 4, 256], bf16, name="scg", tag="scg")
                    nc.scalar.activation(
                        scg[:, 0 : 2 * n, :], psg[:, 0 : 2 * n, :],
                        Act.Abs, accum_out=hcol,
                    )
                else:
                    nc.vector.tensor_reduce(
                        hcol, psg[:, 0 : 2 * n, :], axis=mybir.AxisListType.XY,
                        op=AluOp.add, apply_absolute_value=True,
                    )

            prev = None  # (pair index, n)
            for ki, gi in enumerate(PROC):
                q, lo, n = DMA_GROUPS[gi]
                xt = xts[gi]
                # PE: S = sum_c (a_c - b_c) via 3 DoubleRow matmuls per view
                pss = psSp.tile([128, 2, 512], f32, name="pss", tag="pss")
                for vl in range(n):
                    for c in range(C):
                        nc.tensor.matmul(
                            pss[:, vl, :], wpt[:], xt[:, vl, c],
                            start=(c == 0), stop=(c == C - 1), perf_mode=DR,
                        )
                # gh of the PREVIOUS pair sits here in PE program order,
                # hiding its S-copy latency (software pipelining)
                if prev is not None:
                    emit_gh(*prev)
                pss4 = pss[:].rearrange("p v (s w) -> p v s w", s=2)
                # S copy PSUM f32 -> SBUF bf16 (gh moving operand)
                stp = spool.tile([128, 2, 2, 256], bf16, name=f"st{ki}", tag="st")
                sts[ki] = stp
                if COPY_ENG[ki] == "a":
                    nc.scalar.activation(stp[:, 0:n], pss4[:, 0:n], Act.Copy)
                else:
                    nc.vector.tensor_scalar_add(stp[:, 0:n], pss4[:, 0:n], 0.0)
                # gw = S[., w+1] - S[., w-1]: interior diff + |edge| copies
                # (TensorTensor may read at most one PSUM input, so gw
                # reads the SBUF S-copy)
                gwt = gpool.tile([128, 2, 2, 256], bf16, name="gwt", tag="gwt")
                geng = nc.vector if GWSUB_SRC[ki] == "p" else nc.gpsimd
                geng.tensor_tensor(
                    gwt[:, 0:n, :, 0:254], stp[:, 0:n, :, 2:256],
                    stp[:, 0:n, :, 0:254], AluOp.subtract,
                )
                geng.tensor_scalar_add(
                    gwt[:, 0:n, :, 254:256], stp[:, 0:n, :, 1:255:253], 0.0
                )
                wcol = acc[:, ki : ki + 1]
                if GWABS_ENG[ki] == "a":
                    scr = zpool.tile([128, 2, 2, 256], bf16, name="scr", tag="scr")
                    nc.scalar.activation(
                        scr[:, 0:n], gwt[:, 0:n], Act.Abs, accum_out=wcol,
                    )
                else:
                    nc.vector.tensor_reduce(
                        wcol, gwt[:, 0:n], axis=mybir.AxisListType.XYZ,
                        op=AluOp.add, apply_absolute_value=True,
                    )
                prev = (ki, n)

            emit_gh(*prev)

            nc.sync.dma_start(out=y[:], in_=acc[:])

    nc.finalize()
    _cache["nc"] = nc
    return nc


def _pack(infer, ref):
    """f32 [2,7,7,3,256,256] x2 -> per-core fp8 [128, V, C, 2, 512] packed."""
    f8 = ml_dtypes.float8_e4m3
    a = np.asarray(infer, dtype=np.float32).reshape(98, C, H, W).astype(f8)
    b = np.asarray(ref, dtype=np.float32).reshape(98, C, H, W).astype(f8)
    pad = np.zeros((6, C, H, W), f8)
    a = np.concatenate([a, pad], axis=0).reshape(104, C, 128, 2, W)
    b = np.concatenate([b, pad], axis=0).reshape(104, C, 128, 2, W)
    X = np.stack([a, b], axis=3)                # [104, C, 128, t, s, W]
    X = X.transpose(2, 0, 1, 3, 4, 5)           # [128, 104, C, t, s, W]
    cores = []
    for i in range(N_CORES):
        xi = np.ascontiguousarray(X[:, i * V : (i + 1) * V])
        cores.append(xi.reshape(128, V, C, 2, 512))
    return cores


def _run(infer, ref, trace=False, trace_kwargs=None):
    nc = _build()
    cores = _pack(infer, ref)
    wpair, weo = _weights()
    wpair = wpair.astype(ml_dtypes.float8_e4m3)
    weo = weo.astype(ml_dtypes.bfloat16)
    in_maps = [
        {"x": cores[i], "wp": wpair, "we": weo} for i in range(N_CORES)
    ]
    kwargs = {}
    if trace:
        kwargs["trace"] = True
        if trace_kwargs:
            kwargs["trace_kwargs"] = trace_kwargs
    out = run_bass_kernel_spmd(nc, in_maps, core_ids=list(range(N_CORES)), **kwargs)
    total = 0.0
    for res in out.results:
        total += res["y"].astype(np.float64).sum()
    loss = np.float32(total * SCALE)
    return loss, out


def kernel(infer, ref):
    loss, _ = _run(infer, ref)
    return np.asarray(loss, dtype=np.float32)


# revision 13
# speedup vs baseline: 1.0698x; 1.0698x over previous
"""Detail-loss kernel for TRN2 (8 NeuronCores), v3.

Reference computation (algebraically reduced):
  views = reshape(inputs, (98, 3, 256, 256)); d = infer - ref
  S[n] = sum_c d[n, c]                       (per-view 256x256 plane)
  loss = ( sum |S[n,h,w+1] - S[n,h,w-1]|     (zero-padded outside)
         + sum |S[n,h+1,w] - S[n,h-1,w]| ) / (4 * 98 * 258 * 256)

Sharding: 98 views padded to 104, 13 views per core (zero views add 0).

v3 changes vs v2 (41.8us):
  * DMA rebalanced: views 0-4 on gpsimd(SWDGE), 5-8 on sync, 9-12 on
    scalar queues as 2-view-group transfers (128 x 6KB descriptors).
    v2 put 9/13 views on gpsimd -> 7us single-queue DMA tail.
  * PE warmup: dummy matmuls on junk data at kernel start lift the
    PE_HAM clock gate (1.2 -> 2.4 GHz) before real data arrives, so
    real matmuls run at the 216ns warm cadence instead of ~430ns.
  * gw computed directly from PSUM S on DVE (interior diff + edge-col
    copy), removing the S-copy -> gw chain; S-copy (needed only as the
    gh matmul moving operand) runs in parallel on ACT.
  * per-pair abs-accumulate ops alternate DVE/ACT; some mid-kernel gw
    subtracts route via SBUF on gpsimd to relieve DVE.
Host: sum partials in float64, scale.
"""
import numpy as np
import ml_dtypes
import concourse.bass as bass
import concourse.mybir as mybir
from concourse import bacc
from concourse.tile import TileContext
from concourse.bass_utils import run_bass_kernel_spmd

N_CORES = 8
V = 13                       # views per core (98 -> 104 padded)
C, H, W = 3, 256, 256
SCALE = 1.0 / (4.0 * 98.0 * 258.0 * 256.0)
NPAIR = 7
NCOL = 2 * NPAIR             # 7 gw cols + 7 gh cols

# All x traffic rides ONE DMA ring (gpsimd/SWDGE): a single ring running
# alone sustains ~358GB/s (HBM cap), while concurrent rings round-robin
# at packet granularity and all complete late (~260GB/s aggregate and no
# useful ordering). FIFO per-ring order = processing order. First views
# go per-view for early pipeline start; later ones per-pair (fewer
# descriptor-gen ops on the issuing Q7 engine).
DMA_UNITS = [(0, 1), (1, 1), (2, 1), (3, 1), (4, 2), (6, 2), (8, 2), (10, 2), (12, 1)]
PAIRS = [(0, 2), (2, 2), (4, 2), (6, 2), (8, 2), (10, 2), (12, 1)]

N_WARM = 7                   # PE warmup matmuls (512 cols, plain mode)

# engine assignment per pair: 'a'=ACT(scalar), 'v'=DVE(vector), 'g'=gpsimd
# (gpsimd is busy generating DMA descriptors until ~14us, so early and
# final gw-subs go to DVE)
COPY_ENG = "avavava"
GWSUB_ENG = "vgggggv"
GWABS_ENG = "vavavav"
GHABS_ENG = "avavava"

_cache = {}


def _weights():
    I = np.eye(128, dtype=np.float32)
    E = (np.eye(128) - np.eye(128, k=1)).astype(np.float32)   # out[o]=in[o]-in[o-1]
    O = (np.eye(128, k=-1) - np.eye(128)).astype(np.float32)  # out[o]=in[o+1]-in[o]
    wpair = np.stack([I, -I], axis=1)  # [128, 2, 128] DoubleRow stationary
    weo = np.stack([E, O], axis=1)     # [128, 2, 128]
    return wpair, weo


def _build():
    if "nc" in _cache:
        return _cache["nc"]
    f32 = mybir.dt.float32
    bf16 = mybir.dt.bfloat16
    f8 = mybir.dt.float8e4
    AluOp = mybir.AluOpType
    Act = mybir.ActivationFunctionType
    DR = mybir.MatmulPerfMode.DoubleRow

    nc = bacc.Bacc(None, target_bir_lowering=False)
    x = nc.declare_dram_parameter("x", [128, V, C, 2, 512], f8, isOutput=False)
    wp = nc.declare_dram_parameter("wp", [128, 2, 128], f8, isOutput=False)
    we = nc.declare_dram_parameter("we", [128, 2, 128], bf16, isOutput=False)
    y = nc.declare_dram_parameter("y", [128, NCOL], f32, isOutput=True)

    with TileContext(nc) as tc:
        with (
            tc.tile_pool(name="wpool", bufs=1) as wpool,
            tc.tile_pool(name="jp", bufs=1) as jpool,
            tc.tile_pool(name="xp", bufs=1) as xpool,
            tc.tile_pool(name="sp", bufs=3) as spool,
            tc.tile_pool(name="gp", bufs=2) as gpool,
            tc.tile_pool(name="zp", bufs=2) as zpool,
            tc.tile_pool(name="cp", bufs=2) as cpool,
            tc.tile_pool(name="ap", bufs=1) as apool,
            tc.tile_pool(name="psS", bufs=2, space="PSUM") as psSp,
            tc.tile_pool(name="psG", bufs=2, space="PSUM") as psGp,
        ):
            # ---- junk memset first so PE warmup can start immediately
            junk = jpool.tile([128, 512], f8)
            nc.vector.memset(junk[:], 0.0)

            # ---- weight DMAs on sync (own HWDGE ring, completes before x
            # traffic ramps); all x views on the gpsimd ring in FIFO order
            wpt = wpool.tile([128, 2, 128], f8)
            wet = wpool.tile([128, 2, 128], bf16)
            nc.sync.dma_start(out=wpt[:], in_=wp[:])
            nc.sync.dma_start(out=wet[:], in_=we[:])

            view_src = {}
            for ui, (lo, n) in enumerate(DMA_UNITS):
                xt = xpool.tile([128, n, C, 2, 512], f8, name=f"xu{ui}")
                nc.gpsimd.dma_start(out=xt[:], in_=x[:, lo : lo + n])
                for k in range(n):
                    view_src[lo + k] = (xt, k)

            # ---- PE warmup: plain-mode junk matmuls lift the HAM clock
            # gate (1.2 -> 2.4 GHz) while DMA streams in; the scratch PSUM
            # output is never read.
            warm = psGp.tile([128, 4, 256], f32, name="warm", tag="psg")
            wview = warm[:].rearrange("p a b -> p (a b)")
            for _ in range(N_WARM):
                nc.tensor.matmul(
                    wview[:, 0:512], junk[:, 0:128], junk[:],
                    start=True, stop=True,
                )

            acc = apool.tile([128, NCOL], f32)

            sts = {}   # pair k -> stp tile (SBUF S, gh moving operand)
            pss_of = {}

            def emit_gh(k, n):
                stp = sts[k]
                psg = psGp.tile([128, 4, 256], f32, name="psg", tag="psg")
                nc.tensor.matmul(
                    psg[:, 0:n, :], wet[:, 0, :], stp[:, 0:n, 1, :],
                    start=True, stop=True,
                )
                nc.tensor.matmul(
                    psg[:, n : 2 * n, :], wet[:, 1, :], stp[:, 0:n, 0, :],
                    start=True, stop=True,
                )
                hcol = acc[:, NPAIR + k : NPAIR + k + 1]
                if GHABS_ENG[k] == "a":
                    scg = cpool.tile([128, 4, 256], bf16, name="scg", tag="scg")
                    nc.scalar.activation(
                        scg[:, 0 : 2 * n, :], psg[:, 0 : 2 * n, :],
                        Act.Abs, accum_out=hcol,
                    )
                else:
                    nc.vector.tensor_reduce(
                        hcol, psg[:, 0 : 2 * n, :], axis=mybir.AxisListType.XY,
                        op=AluOp.add, apply_absolute_value=True,
                    )

            prev = None  # (pair index, n)
            for ki, (lo, n) in enumerate(PAIRS):
                # PE: S = sum_c (a_c - b_c) via 3 DoubleRow matmuls per view
                pss = psSp.tile([128, 2, 512], f32, name="pss", tag="pss")
                for vl in range(n):
                    xt, li = view_src[lo + vl]
                    for c in range(C):
                        nc.tensor.matmul(
                            pss[:, vl, :], wpt[:], xt[:, li, c],
                            start=(c == 0), stop=(c == C - 1), perf_mode=DR,
                        )
                # gh of the PREVIOUS pair sits here in PE program order,
                # hiding its S-copy latency (software pipelining)
                if prev is not None:
                    emit_gh(*prev)
                pss4 = pss[:].rearrange("p v (s w) -> p v s w", s=2)
                # S copy PSUM f32 -> SBUF bf16 (gh moving operand)
                stp = spool.tile([128, 2, 2, 256], bf16, name=f"st{ki}", tag="st")
                sts[ki] = stp
                if COPY_ENG[ki] == "a":
                    nc.scalar.activation(stp[:, 0:n], pss4[:, 0:n], Act.Copy)
                else:
                    nc.vector.tensor_scalar_add(stp[:, 0:n], pss4[:, 0:n], 0.0)
                # gw = S[., w+1] - S[., w-1]: interior diff + |edge| copies
                # (TensorTensor may read at most one PSUM input, so gw
                # reads the SBUF S-copy)
                gwt = gpool.tile([128, 2, 2, 256], bf16, name="gwt", tag="gwt")
                geng = nc.vector if GWSUB_ENG[ki] == "v" else nc.gpsimd
                geng.tensor_tensor(
                    gwt[:, 0:n, :, 0:254], stp[:, 0:n, :, 2:256],
                    stp[:, 0:n, :, 0:254], AluOp.subtract,
                )
                geng.tensor_scalar_add(
                    gwt[:, 0:n, :, 254:256], stp[:, 0:n, :, 1:255:253], 0.0
                )
                wcol = acc[:, ki : ki + 1]
                if GWABS_ENG[ki] == "a":
                    scr = zpool.tile([128, 2, 2, 256], bf16, name="scr", tag="scr")
                    nc.scalar.activation(
                        scr[:, 0:n], gwt[:, 0:n], Act.Abs, accum_out=wcol,
                    )
                else:
                    nc.vector.tensor_reduce(
                        wcol, gwt[:, 0:n], axis=mybir.AxisListType.XYZ,
                        op=AluOp.add, apply_absolute_value=True,
                    )
                prev = (ki, n)

            emit_gh(*prev)

            nc.sync.dma_start(out=y[:], in_=acc[:])

    nc.finalize()
    _cache["nc"] = nc
    return nc


def _pack(infer, ref):
    """f32 [2,7,7,3,256,256] x2 -> per-core fp8 [128, V, C, 2, 512] packed."""
    f8 = ml_dtypes.float8_e4m3
    a = np.asarray(infer, dtype=np.float32).reshape(98, C, H, W).astype(f8)
    b = np.asarray(ref, dtype=np.float32).reshape(98, C, H, W).astype(f8)
    pad = np.zeros((6, C, H, W), f8)
    a = np.concatenate([a, pad], axis=0).reshape(104, C, 128, 2, W)
    b = np.concatenate([b, pad], axis=0).reshape(104, C, 128, 2, W)
    X = np.stack([a, b], axis=3)                # [104, C, 128, t, s, W]
    X = X.transpose(2, 0, 1, 3, 4, 5)           # [128, 104, C, t, s, W]
    cores = []
    for i in range(N_CORES):
        xi = np.ascontiguousarray(X[:, i * V : (i + 1) * V])
        cores.append(xi.reshape(128, V, C, 2, 512))
    return cores


def _run(infer, ref, trace=False, trace_kwargs=None):
    nc = _build()
    cores = _pack(infer, ref)
    wpair, weo = _weights()
    wpair = wpair.astype(ml_dtypes.float8_e4m3)
    weo = weo.astype(ml_dtypes.bfloat16)
    in_maps = [
        {"x": cores[i], "wp": wpair, "we": weo} for i in range(N_CORES)
    ]
    kwargs = {}
    if trace:
        kwargs["trace"] = True
        if trace_kwargs:
            kwargs["trace_kwargs"] = trace_kwargs
    out = run_bass_kernel_spmd(nc, in_maps, core_ids=list(range(N_CORES)), **kwargs)
    total = 0.0
    for res in out.results:
        total += res["y"].astype(np.float64).sum()
    loss = np.float32(total * SCALE)
    return loss, out


def kernel(infer, ref):
    loss, _ = _run(infer, ref)
    return np.asarray(loss, dtype=np.float32)
